# revision 36
# baseline (speedup 1.0000x reference)
"""Trainium2 Bass kernel for 16-head MultiHeadAttention (v3).

Problem shapes (hardcoded): B=2, L=2048, D=1024, H=16, DK=64, fp32 I/O.

Sharding over 8 cores: core c handles batch b=c//4 and head-group g=c%4
(4 heads, 256 of the 1024 QKV columns).  All matmul operands are bf16
(f32 PSUM accumulation); X/weights are cast to bf16 on the host.

Per core:
  X^T via DMA-XBAR transpose straight from HBM           [128,8dc,2048]
  QT/KT = W^T X^T + b  (ACT Identity+bias writes bf16)   [128,2m,2048]
  Vaug  = X Wvaug + bvaug ([1 | V] per head: ones col FIRST so the
          softmax denominator lands on PSUM partition 0) [128,16lt,260]
  attention per l-half (1024 q-cols), per head, per kti: S^T (2 matmuls
  sharing the kt lhsT) -> exp (ACT, scale 1/8) -> AV accumulate
  [65,1024] (row 0 = denominator)
  normalize: recip(row0) -> gpsimd partition_broadcast -> mul (reads
  PSUM directly, evacuating it)
  pack: gpsimd SWDGE DMA casts f32->bf16 into ain[half]; AllToAll per
  half ships rows 128s (of the half) to core s (512KB bf16)
  Y[128,1024] = oall^T Wo + bo per (half, batch); core c outputs rows
  {128c, 1024+128c}.
"""

import numpy as np
import ml_dtypes
from contextlib import ExitStack

import concourse.bass as bass
import concourse.bacc as bacc
import concourse.mybir as mybir
import concourse.tile as tile
from concourse.bass_utils import run_bass_kernel_spmd

F32 = mybir.dt.float32
BF16 = mybir.dt.bfloat16
AF = mybir.ActivationFunctionType

B, L, D, H, DK = 2, 2048, 1024, 16, 64
NCORES = 8
NH = 4              # heads per core
CPC = NH * DK       # 256 qkv cols per core
VA = NH * (DK + 1)  # 260, V-aug width (per-head [1 | V])
LT = L // 128       # 16 k-chunks
DCH = D // 128      # 8 d-chunks
NQ = 4              # l-quarters (projection granularity)
QW = L // NQ        # 512


def _emit(tc, nc, x, wq, bq, wk, bk, wv, bv, wo, bo, idm, out):
    with ExitStack() as es:
        # ---------------- persistent pools ----------------
        const = es.enter_context(tc.tile_pool(name="const", bufs=1))
        wq_sb = const.tile([128, DCH, CPC], BF16)
        wk_sb = const.tile([128, DCH, CPC], BF16)
        wv_sb = const.tile([128, DCH, VA], BF16)
        wo_sb = const.tile([128, DCH, D], BF16)
        bq_sb = const.tile([128, 2, 1], F32)
        bk_sb = const.tile([128, 2, 1], F32)
        bv_bc = const.tile([128, VA], F32)
        bo_bc = const.tile([128, D], F32)

        proj = es.enter_context(tc.tile_pool(name="proj", bufs=1))
        qt = proj.tile([128, 2, L], BF16)       # Q^T, c-chunk m rows
        kt = proj.tile([128, 2, L], BF16)       # K^T
        vaug = proj.tile([128, LT, VA], BF16)   # [1 | V] per k-chunk

        # ---------------- phase A: X^T + projections ----------------
        with ExitStack() as phA:
            xtp = phA.enter_context(tc.tile_pool(name="xtp", bufs=1))
            xt = xtp.tile([128, DCH, L], BF16)  # X^T (d on partitions)
            ident = xtp.tile([128, 128], BF16)
            xload = phA.enter_context(tc.tile_pool(name="xload", bufs=2))
            pst = phA.enter_context(tc.tile_pool(name="pst", bufs=2,
                                                 space="PSUM"))
            psA = phA.enter_context(tc.tile_pool(name="psA", bufs=3,
                                                 space="PSUM"))
            psV = phA.enter_context(tc.tile_pool(name="psV", bufs=2,
                                                 space="PSUM"))

            # identity (host-supplied) + Q/K weights first on the
            # scalar queue so transposes/proj are never weight-blocked.
            nc.scalar.dma_start(ident, idm.ap())
            nc.scalar.dma_start(
                wk_sb, wk.ap().rearrange("(dc p) c -> p dc c", p=128))
            nc.scalar.dma_start(
                wq_sb, wq.ap().rearrange("(dc p) c -> p dc c", p=128))
            nc.scalar.dma_start(
                wv_sb, wv.ap().rearrange("(dc p) c -> p dc c", p=128))
            nc.scalar.dma_start(
                bk_sb, bk.ap().rearrange("(m p) o -> p m o", p=128))
            nc.scalar.dma_start(
                bq_sb, bq.ap().rearrange("(m p) o -> p m o", p=128))
            # broadcast loads (stride-0 partition reads) on SWDGE
            bv_ap = bv.ap()
            nc.gpsimd.dma_start(
                bv_bc, bass.AP(tensor=bv_ap.tensor, offset=bv_ap.offset,
                               ap=[[0, 128]] + list(bv_ap.ap[1:])))
            bo_ap = bo.ap()
            nc.gpsimd.dma_start(
                bo_bc, bass.AP(tensor=bo_ap.tensor, offset=bo_ap.offset,
                               ap=[[0, 128]] + list(bo_ap.ap[1:])))

            # X^T via PE transposes, per 128-row chunk of X
            xap = x.ap()

            def emit_xt(lt):
                xrow = xload.tile([128, D], BF16, tag="xrow",
                                  name=f"xrow{lt}")
                nc.sync.dma_start(xrow, xap[lt * 128:(lt + 1) * 128, :])
                for dg in range(2):
                    pt = pst.tile([128, 512], BF16, tag="pt",
                                  name=f"pt{lt}{dg}")
                    for q in range(4):
                        dc = dg * 4 + q
                        nc.tensor.transpose(
                            pt[:, q * 128:(q + 1) * 128],
                            xrow[:, dc * 128:(dc + 1) * 128], ident)
                    nc.vector.tensor_copy(
                        xt[:, dg * 4:dg * 4 + 4, lt * 128:(lt + 1) * 128],
                        pt.rearrange("p (a b) -> p a b", a=4))

            # Wo is only needed for Y at the very end
            nc.scalar.dma_start(
                wo_sb, wo.ap().rearrange("(j p) n -> p j n", p=128))

            # K^T / Q^T projections per l-quarter (K first: attention's
            # lhsT).  Bias-add + bf16 cast on the otherwise-idle ACT.
            for lq in range(NQ):
                for lt in range(lq * 4, lq * 4 + 4):
                    emit_xt(lt)
                for w_sb, b_sb, dst in ((wk_sb, bk_sb, kt),
                                        (wq_sb, bq_sb, qt)):
                    for m in range(2):
                        pq = psA.tile([128, QW], F32, tag="pq")
                        for dc in range(DCH):
                            nc.tensor.matmul(
                                pq,
                                w_sb[:, dc, m * 128:(m + 1) * 128],
                                xt[:, dc, lq * QW:(lq + 1) * QW],
                                start=(dc == 0), stop=(dc == DCH - 1))
                        nc.scalar.activation(
                            dst[:, m, lq * QW:(lq + 1) * QW], pq,
                            AF.Identity, bias=b_sb[:, m, :])
                # V projection for this quarter's 4 k-chunks (DVE adds
                # the tensor-valued bias and casts to bf16)
                for lt in range(lq * 4, lq * 4 + 4):
                    pv = psV.tile([128, VA], F32, tag="pv")
                    for dc in range(DCH):
                        nc.tensor.matmul(
                            pv, xt[:, dc, lt * 128:(lt + 1) * 128],
                            wv_sb[:, dc, :],
                            start=(dc == 0), stop=(dc == DCH - 1))
                    nc.vector.tensor_add(vaug[:, lt, :], pv, bv_bc)

        # ---------------- phase B1: attention + exchange ----------------
        dramp = es.enter_context(tc.tile_pool(name="dramp", bufs=1,
                                              space="DRAM"))
        ain = dramp.tile([2, NCORES, CPC, 128], BF16)
        aout = dramp.tile([2, NCORES, CPC, 128], BF16)

        with ExitStack() as phB:
            psS = phB.enter_context(tc.tile_pool(name="psS", bufs=3,
                                                 space="PSUM"))
            psO = phB.enter_context(tc.tile_pool(name="psO", bufs=1,
                                                 space="PSUM"))
            upool = phB.enter_context(tc.tile_pool(name="upool", bufs=6))
            npool = phB.enter_context(tc.tile_pool(name="npool", bufs=2))

            for half in range(2):
                q0 = half * 1024
                for h in range(NH):
                    m, r0 = h // 2, (h % 2) * 64
                    otp = psO.tile([65, 1024], F32, tag="otp",
                                   name=f"otp{half}{h}")
                    for kti in range(LT):
                        sp = psS.tile([128, 1024], F32, tag="sp",
                                      name=f"sp{half}{h}{kti}")
                        for nn in range(2):
                            nc.tensor.matmul(
                                sp[:, nn * 512:(nn + 1) * 512],
                                kt[r0:r0 + 64, m,
                                   kti * 128:(kti + 1) * 128],
                                qt[r0:r0 + 64, m,
                                   q0 + nn * 512:q0 + (nn + 1) * 512],
                                start=True, stop=True)
                        u = upool.tile([128, 1024], BF16, tag="u",
                                       name=f"u{half}{h}{kti}")
                        nc.scalar.activation(u, sp, AF.Exp, scale=0.125)
                        for nn in range(2):
                            nc.tensor.matmul(
                                otp[:, nn * 512:(nn + 1) * 512],
                                vaug[:, kti, h * 65:(h + 1) * 65],
                                u[:, nn * 512:(nn + 1) * 512],
                                start=(kti == 0), stop=(kti == LT - 1))
                    # normalization: row 0 of otp is the denominator.
                    # Evacuate PSUM immediately (psO bufs=1), recip the
                    # denominator row, gpsimd broadcast, multiply.
                    otu = npool.tile([65, 1024], F32, tag="otu",
                                     name=f"otu{half}{h}")
                    nc.vector.tensor_copy(otu, otp)
                    rec1 = npool.tile([1, 1024], F32, tag="rec1",
                                      name=f"rec1{half}{h}")
                    nc.vector.reciprocal_approx_fast(rec1, otu[0:1, :])
                    rbc = npool.tile([65, 1024], F32, tag="rbc",
                                     name=f"rbc{half}{h}")
                    nc.gpsimd.partition_broadcast(rbc, rec1, channels=65)
                    otn = npool.tile([65, 1024], BF16, tag="otn",
                                     name=f"otn{half}{h}")
                    nc.vector.tensor_mul(otn, otu, rbc)
                    # pack into ain[half] (plain bf16 copy)
                    dst = ain[half, :, h * 64:(h + 1) * 64, :].rearrange(
                        "s p l -> p s l")
                    nc.sync.dma_start(
                        dst, otn[1:65, :].rearrange("p (s l) -> p s l",
                                                    s=NCORES))
                nc.gpsimd.collective_compute(
                    "AllToAll", mybir.AluOpType.bypass,
                    replica_groups=[list(range(NCORES))],
                    ins=[ain[half].opt()], outs=[aout[half].opt()])

        # ---------------- phase B2: output Y ----------------
        with ExitStack() as phC:
            psY = phC.enter_context(tc.tile_pool(name="psY", bufs=4,
                                                 space="PSUM"))
            opool = phC.enter_context(tc.tile_pool(name="opool", bufs=2))
            ypool = phC.enter_context(tc.tile_pool(name="ypool", bufs=2))
            outap = out.ap()
            for half in range(2):
                for b in range(B):
                    oall = opool.tile([128, DCH, 128], BF16, tag="oall",
                                      name=f"oall{half}{b}")
                    src = aout[half, 4 * b:4 * b + 4, :, :]
                    nc.sync.dma_start(
                        oall,
                        src.rearrange("s (jl p) l -> p (s jl) l", p=128))
                    y_sb = ypool.tile([128, D], F32, tag="ysb",
                                      name=f"ysb{half}{b}")
                    yp0 = psY.tile([128, 512], F32, tag="yp",
                                   name=f"yp{half}{b}0")
                    yp1 = psY.tile([128, 512], F32, tag="yp",
                                   name=f"yp{half}{b}1")
                    for j in range(DCH):
                        for nn, yp in ((0, yp0), (1, yp1)):
                            nc.tensor.matmul(
                                yp, oall[:, j, :],
                                wo_sb[:, j, nn * 512:(nn + 1) * 512],
                                start=(j == 0), stop=(j == DCH - 1))
                    for nn, yp in ((0, yp0), (1, yp1)):
                        nc.vector.tensor_add(
                            y_sb[:, nn * 512:(nn + 1) * 512], yp,
                            bo_bc[:, nn * 512:(nn + 1) * 512])
                    nc.sync.dma_start(outap[b, half, :, :], y_sb)


_CACHED_NC = None


def _build_program():
    global _CACHED_NC
    if _CACHED_NC is not None:
        return _CACHED_NC
    nc = bacc.Bacc(None, target_bir_lowering=False, debug=False,
                   num_devices=NCORES)
    x = nc.declare_dram_parameter("x", [L, D], BF16, isOutput=False)
    wq = nc.declare_dram_parameter("wq", [D, CPC], BF16, isOutput=False)
    bq = nc.declare_dram_parameter("bq", [CPC, 1], F32, isOutput=False)
    wk = nc.declare_dram_parameter("wk", [D, CPC], BF16, isOutput=False)
    bk = nc.declare_dram_parameter("bk", [CPC, 1], F32, isOutput=False)
    wv = nc.declare_dram_parameter("wv", [D, VA], BF16, isOutput=False)
    bv = nc.declare_dram_parameter("bv", [1, VA], F32, isOutput=False)
    wo = nc.declare_dram_parameter("wo", [D, D], BF16, isOutput=False)
    bo = nc.declare_dram_parameter("bo", [1, D], F32, isOutput=False)
    idm = nc.declare_dram_parameter("idm", [128, 128], BF16,
                                    isOutput=False)
    out = nc.declare_dram_parameter("out", [B, 2, 128, D], F32,
                                    isOutput=True)

    with tile.TileContext(nc) as tc:
        _emit(tc, nc, x, wq, bq, wk, bk, wv, bv, wo, bo, idm, out)
    nc.finalize()
    _CACHED_NC = nc
    return nc


def _make_in_maps(X, WQ, bQ, WK, bK, WV, bV, WO, bO):
    bf = ml_dtypes.bfloat16
    X = np.asarray(X, np.float32)
    WQ = np.asarray(WQ, np.float32)
    WK = np.asarray(WK, np.float32)
    WV = np.asarray(WV, np.float32)
    WO = np.ascontiguousarray(np.asarray(WO, np.float32).astype(bf))
    bO = np.asarray(bO, np.float32).reshape(1, D)
    in_maps = []
    for c in range(NCORES):
        b, g = c // 4, c % 4
        cs = slice(CPC * g, CPC * (g + 1))
        wva = np.zeros((D, VA), np.float32)
        bva = np.zeros((1, VA), np.float32)
        for h in range(NH):
            # ones column FIRST, then the 64 V columns
            bva[0, h * 65] = 1.0
            wva[:, h * 65 + 1:h * 65 + 65] = WV[:, CPC * g + 64 * h:
                                                CPC * g + 64 * (h + 1)]
            bva[0, h * 65 + 1:h * 65 + 65] = bV[CPC * g + 64 * h:
                                                CPC * g + 64 * (h + 1)]
        in_maps.append({
            "x": np.ascontiguousarray(X[b].astype(bf)),
            "wq": np.ascontiguousarray(WQ[:, cs].astype(bf)),
            "bq": np.ascontiguousarray(np.asarray(bQ, np.float32)[cs]
                                       .reshape(CPC, 1)),
            "wk": np.ascontiguousarray(WK[:, cs].astype(bf)),
            "bk": np.ascontiguousarray(np.asarray(bK, np.float32)[cs]
                                       .reshape(CPC, 1)),
            "wv": np.ascontiguousarray(wva.astype(bf)),
            "bv": bva,
            "wo": WO,
            "bo": np.ascontiguousarray(bO),
            "idm": np.eye(128, dtype=bf),
        })
    return in_maps


def _assemble(results):
    full = np.empty((B, L, D), np.float32)
    for c in range(NCORES):
        o = results[c]["out"]  # [B, 2, 128, D]
        for b in range(B):
            full[b, 128 * c:128 * (c + 1), :] = o[b, 0]
            full[b, 1024 + 128 * c:1024 + 128 * (c + 1), :] = o[b, 1]
    return full


def run(inputs, trace=False):
    nc = _build_program()
    in_maps = _make_in_maps(**inputs)
    res = run_bass_kernel_spmd(nc, in_maps, list(range(NCORES)), trace=trace)
    return _assemble(res.results), res


def kernel(X, WQ, bQ, WK, bK, WV, bV, WO, bO):
    out, _ = run(dict(X=X, WQ=WQ, bQ=bQ, WK=WK, bK=bK, WV=WV, bV=bV,
                      WO=WO, bO=bO))
    return out


# revision 38
# speedup vs baseline: 1.0236x; 1.0236x over previous
"""Trainium2 Bass kernel for 16-head MultiHeadAttention (v3).

Problem shapes (hardcoded): B=2, L=2048, D=1024, H=16, DK=64, fp32 I/O.

Sharding over 8 cores: core c handles batch b=c//4 and head-group g=c%4
(4 heads, 256 of the 1024 QKV columns).  All matmul operands are bf16
(f32 PSUM accumulation); X/weights are cast to bf16 on the host.

Per core:
  X^T via DMA-XBAR transpose straight from HBM           [128,8dc,2048]
  QT/KT = W^T X^T + b  (ACT Identity+bias writes bf16)   [128,2m,2048]
  Vaug  = X Wvaug + bvaug ([1 | V] per head: ones col FIRST so the
          softmax denominator lands on PSUM partition 0) [128,16lt,260]
  attention per l-half (1024 q-cols), per head, per kti: S^T (2 matmuls
  sharing the kt lhsT) -> exp (ACT, scale 1/8) -> AV accumulate
  [65,1024] (row 0 = denominator)
  normalize: recip(row0) -> gpsimd partition_broadcast -> mul (reads
  PSUM directly, evacuating it)
  pack: gpsimd SWDGE DMA casts f32->bf16 into ain[half]; AllToAll per
  half ships rows 128s (of the half) to core s (512KB bf16)
  Y[128,1024] = oall^T Wo + bo per (half, batch); core c outputs rows
  {128c, 1024+128c}.
"""

import numpy as np
import ml_dtypes
from contextlib import ExitStack

import concourse.bass as bass
import concourse.bacc as bacc
import concourse.mybir as mybir
import concourse.tile as tile
from concourse.bass_utils import run_bass_kernel_spmd

F32 = mybir.dt.float32
BF16 = mybir.dt.bfloat16
AF = mybir.ActivationFunctionType

B, L, D, H, DK = 2, 2048, 1024, 16, 64
NCORES = 8
NH = 4              # heads per core
CPC = NH * DK       # 256 qkv cols per core
VA = NH * (DK + 1)  # 260, V-aug width (per-head [1 | V])
LT = L // 128       # 16 k-chunks
DCH = D // 128      # 8 d-chunks
NQ = 4              # l-quarters (projection granularity)
QW = L // NQ        # 512


def _emit(tc, nc, x, wq, bq, wk, bk, wv, bv, wo, bo, idm, out):
    with ExitStack() as es:
        # ---------------- persistent pools ----------------
        const = es.enter_context(tc.tile_pool(name="const", bufs=1))
        wq_sb = const.tile([128, DCH, CPC], BF16)
        wk_sb = const.tile([128, DCH, CPC], BF16)
        wv_sb = const.tile([128, DCH, VA], BF16)
        wo_sb = const.tile([128, DCH, D], BF16)
        bq_sb = const.tile([128, 2, 1], F32)
        bk_sb = const.tile([128, 2, 1], F32)
        bv_bc = const.tile([128, VA], F32)
        bo_bc = const.tile([128, D], F32)

        proj = es.enter_context(tc.tile_pool(name="proj", bufs=1))
        qt = proj.tile([128, 2, L], BF16)       # Q^T, c-chunk m rows
        kt = proj.tile([128, 2, L], BF16)       # K^T
        vaug = proj.tile([128, LT, VA], BF16)   # [1 | V] per k-chunk

        # ---------------- phase A: X^T + projections ----------------
        with ExitStack() as phA:
            xtp = phA.enter_context(tc.tile_pool(name="xtp", bufs=1))
            xt = xtp.tile([128, DCH, L], BF16)  # X^T (d on partitions)
            ident = xtp.tile([128, 128], BF16)
            xload = phA.enter_context(tc.tile_pool(name="xload", bufs=2))
            pst = phA.enter_context(tc.tile_pool(name="pst", bufs=2,
                                                 space="PSUM"))
            psA = phA.enter_context(tc.tile_pool(name="psA", bufs=3,
                                                 space="PSUM"))
            psV = phA.enter_context(tc.tile_pool(name="psV", bufs=2,
                                                 space="PSUM"))

            # identity (host-supplied) + Q/K weights first on the
            # scalar queue so transposes/proj are never weight-blocked.
            nc.scalar.dma_start(ident, idm.ap())
            nc.scalar.dma_start(
                wk_sb, wk.ap().rearrange("(dc p) c -> p dc c", p=128))
            nc.scalar.dma_start(
                wq_sb, wq.ap().rearrange("(dc p) c -> p dc c", p=128))
            nc.scalar.dma_start(
                wv_sb, wv.ap().rearrange("(dc p) c -> p dc c", p=128))
            nc.scalar.dma_start(
                bk_sb, bk.ap().rearrange("(m p) o -> p m o", p=128))
            nc.scalar.dma_start(
                bq_sb, bq.ap().rearrange("(m p) o -> p m o", p=128))
            # broadcast loads (stride-0 partition reads) on SWDGE
            bv_ap = bv.ap()
            nc.gpsimd.dma_start(
                bv_bc, bass.AP(tensor=bv_ap.tensor, offset=bv_ap.offset,
                               ap=[[0, 128]] + list(bv_ap.ap[1:])))
            bo_ap = bo.ap()
            nc.gpsimd.dma_start(
                bo_bc, bass.AP(tensor=bo_ap.tensor, offset=bo_ap.offset,
                               ap=[[0, 128]] + list(bo_ap.ap[1:])))

            # X^T via PE transposes, per 128-row chunk of X
            xap = x.ap()

            def emit_xt(lt):
                xrow = xload.tile([128, D], BF16, tag="xrow",
                                  name=f"xrow{lt}")
                nc.sync.dma_start(xrow, xap[lt * 128:(lt + 1) * 128, :])
                for dg in range(2):
                    pt = pst.tile([128, 512], BF16, tag="pt",
                                  name=f"pt{lt}{dg}")
                    for q in range(4):
                        dc = dg * 4 + q
                        nc.tensor.transpose(
                            pt[:, q * 128:(q + 1) * 128],
                            xrow[:, dc * 128:(dc + 1) * 128], ident)
                    nc.vector.tensor_copy(
                        xt[:, dg * 4:dg * 4 + 4, lt * 128:(lt + 1) * 128],
                        pt.rearrange("p (a b) -> p a b", a=4))

            # Wo is only needed for Y at the very end
            nc.scalar.dma_start(
                wo_sb, wo.ap().rearrange("(j p) n -> p j n", p=128))

            # K^T / Q^T projections per l-quarter (K first: attention's
            # lhsT).  Bias-add + bf16 cast on the otherwise-idle ACT.
            for lq in range(NQ):
                for lt in range(lq * 4, lq * 4 + 4):
                    emit_xt(lt)
                for w_sb, b_sb, dst in ((wk_sb, bk_sb, kt),
                                        (wq_sb, bq_sb, qt)):
                    for m in range(2):
                        pq = psA.tile([128, QW], F32, tag="pq")
                        for dc in range(DCH):
                            nc.tensor.matmul(
                                pq,
                                w_sb[:, dc, m * 128:(m + 1) * 128],
                                xt[:, dc, lq * QW:(lq + 1) * QW],
                                start=(dc == 0), stop=(dc == DCH - 1))
                        nc.scalar.activation(
                            dst[:, m, lq * QW:(lq + 1) * QW], pq,
                            AF.Identity, bias=b_sb[:, m, :])
                # V projection for this quarter's 4 k-chunks (DVE adds
                # the tensor-valued bias and casts to bf16)
                for lt in range(lq * 4, lq * 4 + 4):
                    pv = psV.tile([128, VA], F32, tag="pv")
                    for dc in range(DCH):
                        nc.tensor.matmul(
                            pv, xt[:, dc, lt * 128:(lt + 1) * 128],
                            wv_sb[:, dc, :],
                            start=(dc == 0), stop=(dc == DCH - 1))
                    nc.vector.tensor_add(vaug[:, lt, :], pv, bv_bc)

        # ---------------- phase B1: attention + exchange ----------------
        dramp = es.enter_context(tc.tile_pool(name="dramp", bufs=1,
                                              space="DRAM"))
        ain = dramp.tile([2, NCORES, CPC, 128], BF16)
        aout = dramp.tile([2, NCORES, CPC, 128], BF16)

        with ExitStack() as phB:
            psS = phB.enter_context(tc.tile_pool(name="psS", bufs=2,
                                                 space="PSUM"))
            psO = phB.enter_context(tc.tile_pool(name="psO", bufs=2,
                                                 space="PSUM"))
            upool = phB.enter_context(tc.tile_pool(name="upool", bufs=4))
            npool = phB.enter_context(tc.tile_pool(name="npool", bufs=2))

            for half in range(2):
                q0 = half * 1024
                for h in range(NH):
                    m, r0 = h // 2, (h % 2) * 64
                    otp = psO.tile([65, 1024], F32, tag="otp",
                                   name=f"otp{half}{h}")
                    for kti in range(LT):
                        sp = psS.tile([128, 1024], F32, tag="sp",
                                      name=f"sp{half}{h}{kti}")
                        for nn in range(2):
                            nc.tensor.matmul(
                                sp[:, nn * 512:(nn + 1) * 512],
                                kt[r0:r0 + 64, m,
                                   kti * 128:(kti + 1) * 128],
                                qt[r0:r0 + 64, m,
                                   q0 + nn * 512:q0 + (nn + 1) * 512],
                                start=True, stop=True)
                        u = upool.tile([128, 1024], BF16, tag="u",
                                       name=f"u{half}{h}{kti}")
                        nc.scalar.activation(u, sp, AF.Exp, scale=0.125)
                        for nn in range(2):
                            nc.tensor.matmul(
                                otp[:, nn * 512:(nn + 1) * 512],
                                vaug[:, kti, h * 65:(h + 1) * 65],
                                u[:, nn * 512:(nn + 1) * 512],
                                start=(kti == 0), stop=(kti == LT - 1))
                    # normalization: row 0 of otp is the denominator.
                    # recip on partition 0, gpsimd broadcast, multiply
                    # straight out of PSUM (evacuates otp).
                    rec1 = npool.tile([1, 1024], F32, tag="rec1",
                                      name=f"rec1{half}{h}")
                    nc.vector.reciprocal_approx_fast(rec1, otp[0:1, :])
                    rbc = npool.tile([65, 1024], F32, tag="rbc",
                                     name=f"rbc{half}{h}")
                    nc.gpsimd.partition_broadcast(rbc, rec1, channels=65)
                    otn = npool.tile([65, 1024], BF16, tag="otn",
                                     name=f"otn{half}{h}")
                    nc.vector.tensor_mul(otn, otp, rbc)
                    # pack into ain[half] (plain bf16 copy)
                    dst = ain[half, :, h * 64:(h + 1) * 64, :].rearrange(
                        "s p l -> p s l")
                    nc.sync.dma_start(
                        dst, otn[1:65, :].rearrange("p (s l) -> p s l",
                                                    s=NCORES))
                nc.gpsimd.collective_compute(
                    "AllToAll", mybir.AluOpType.bypass,
                    replica_groups=[list(range(NCORES))],
                    ins=[ain[half].opt()], outs=[aout[half].opt()])

        # ---------------- phase B2: output Y ----------------
        with ExitStack() as phC:
            psY = phC.enter_context(tc.tile_pool(name="psY", bufs=4,
                                                 space="PSUM"))
            opool = phC.enter_context(tc.tile_pool(name="opool", bufs=2))
            ypool = phC.enter_context(tc.tile_pool(name="ypool", bufs=2))
            outap = out.ap()
            for half in range(2):
                for b in range(B):
                    oall = opool.tile([128, DCH, 128], BF16, tag="oall",
                                      name=f"oall{half}{b}")
                    src = aout[half, 4 * b:4 * b + 4, :, :]
                    nc.sync.dma_start(
                        oall,
                        src.rearrange("s (jl p) l -> p (s jl) l", p=128))
                    y_sb = ypool.tile([128, D], F32, tag="ysb",
                                      name=f"ysb{half}{b}")
                    yp0 = psY.tile([128, 512], F32, tag="yp",
                                   name=f"yp{half}{b}0")
                    yp1 = psY.tile([128, 512], F32, tag="yp",
                                   name=f"yp{half}{b}1")
                    for j in range(DCH):
                        for nn, yp in ((0, yp0), (1, yp1)):
                            nc.tensor.matmul(
                                yp, oall[:, j, :],
                                wo_sb[:, j, nn * 512:(nn + 1) * 512],
                                start=(j == 0), stop=(j == DCH - 1))
                    for nn, yp in ((0, yp0), (1, yp1)):
                        nc.vector.tensor_add(
                            y_sb[:, nn * 512:(nn + 1) * 512], yp,
                            bo_bc[:, nn * 512:(nn + 1) * 512])
                    nc.sync.dma_start(outap[b, half, :, :], y_sb)


_CACHED_NC = None


def _build_program():
    global _CACHED_NC
    if _CACHED_NC is not None:
        return _CACHED_NC
    nc = bacc.Bacc(None, target_bir_lowering=False, debug=False,
                   num_devices=NCORES)
    x = nc.declare_dram_parameter("x", [L, D], BF16, isOutput=False)
    wq = nc.declare_dram_parameter("wq", [D, CPC], BF16, isOutput=False)
    bq = nc.declare_dram_parameter("bq", [CPC, 1], F32, isOutput=False)
    wk = nc.declare_dram_parameter("wk", [D, CPC], BF16, isOutput=False)
    bk = nc.declare_dram_parameter("bk", [CPC, 1], F32, isOutput=False)
    wv = nc.declare_dram_parameter("wv", [D, VA], BF16, isOutput=False)
    bv = nc.declare_dram_parameter("bv", [1, VA], F32, isOutput=False)
    wo = nc.declare_dram_parameter("wo", [D, D], BF16, isOutput=False)
    bo = nc.declare_dram_parameter("bo", [1, D], F32, isOutput=False)
    idm = nc.declare_dram_parameter("idm", [128, 128], BF16,
                                    isOutput=False)
    out = nc.declare_dram_parameter("out", [B, 2, 128, D], F32,
                                    isOutput=True)

    with tile.TileContext(nc) as tc:
        _emit(tc, nc, x, wq, bq, wk, bk, wv, bv, wo, bo, idm, out)
    nc.finalize()
    _CACHED_NC = nc
    return nc


def _make_in_maps(X, WQ, bQ, WK, bK, WV, bV, WO, bO):
    bf = ml_dtypes.bfloat16
    X = np.asarray(X, np.float32)
    WQ = np.asarray(WQ, np.float32)
    WK = np.asarray(WK, np.float32)
    WV = np.asarray(WV, np.float32)
    WO = np.ascontiguousarray(np.asarray(WO, np.float32).astype(bf))
    bO = np.asarray(bO, np.float32).reshape(1, D)
    in_maps = []
    for c in range(NCORES):
        b, g = c // 4, c % 4
        cs = slice(CPC * g, CPC * (g + 1))
        wva = np.zeros((D, VA), np.float32)
        bva = np.zeros((1, VA), np.float32)
        for h in range(NH):
            # ones column FIRST, then the 64 V columns
            bva[0, h * 65] = 1.0
            wva[:, h * 65 + 1:h * 65 + 65] = WV[:, CPC * g + 64 * h:
                                                CPC * g + 64 * (h + 1)]
            bva[0, h * 65 + 1:h * 65 + 65] = bV[CPC * g + 64 * h:
                                                CPC * g + 64 * (h + 1)]
        in_maps.append({
            "x": np.ascontiguousarray(X[b].astype(bf)),
            "wq": np.ascontiguousarray(WQ[:, cs].astype(bf)),
            "bq": np.ascontiguousarray(np.asarray(bQ, np.float32)[cs]
                                       .reshape(CPC, 1)),
            "wk": np.ascontiguousarray(WK[:, cs].astype(bf)),
            "bk": np.ascontiguousarray(np.asarray(bK, np.float32)[cs]
                                       .reshape(CPC, 1)),
            "wv": np.ascontiguousarray(wva.astype(bf)),
            "bv": bva,
            "wo": WO,
            "bo": np.ascontiguousarray(bO),
            "idm": np.eye(128, dtype=bf),
        })
    return in_maps


def _assemble(results):
    full = np.empty((B, L, D), np.float32)
    for c in range(NCORES):
        o = results[c]["out"]  # [B, 2, 128, D]
        for b in range(B):
            full[b, 128 * c:128 * (c + 1), :] = o[b, 0]
            full[b, 1024 + 128 * c:1024 + 128 * (c + 1), :] = o[b, 1]
    return full


def run(inputs, trace=False):
    nc = _build_program()
    in_maps = _make_in_maps(**inputs)
    res = run_bass_kernel_spmd(nc, in_maps, list(range(NCORES)), trace=trace)
    return _assemble(res.results), res


def kernel(X, WQ, bQ, WK, bK, WV, bV, WO, bO):
    out, _ = run(dict(X=X, WQ=WQ, bQ=bQ, WK=WK, bK=bK, WV=WV, bV=bV,
                      WO=WO, bO=bO))
    return out


# revision 40
# speedup vs baseline: 1.3051x; 1.2750x over previous
"""Trainium2 Bass kernel for 16-head MultiHeadAttention.

Problem shapes (hardcoded): B=2, L=2048, D=1024, H=16, DK=64, fp32 I/O.

Sharding over 8 cores: core c handles batch b=c//4 and head-group g=c%4
(4 heads, 256 of the 1024 QKV columns).  All matmul operands are bf16
(f32 PSUM accumulation); X/weights and a 128x128 identity are cast /
supplied in bf16 by the host.

Per core:
  X^T via PE transposes (per 128-row chunk of X)         [128,8dc,2048]
  QT/KT = W^T X^T + b  (ACT Identity+bias writes bf16)   [128,2m,2048]
  Vaug  = X Wvaug + bvaug ([1 | V] per head: ones col FIRST so the
          softmax denominator lands on PSUM partition 0) [128,16lt,260]
  attention per l-half (1024 q-cols), per head, per kti: S^T (2 matmuls
  sharing the kt lhsT) -> exp (ACT, scale 1/8) -> AV accumulate
  [65,1024] (row 0 = denominator)
  normalize: recip(denominator row) -> gpsimd partition_broadcast ->
  multiply straight out of PSUM (evacuates otp)
  pack: sync-queue DMA into ain[half] (otn already bf16); one AllToAll
  per half ships rows 128s (of the half) to core s (512KB bf16); the
  gpsimd queue carries ONLY collectives so they never contend
  Y[128,1024] = oall^T Wo + bo per (half, batch); core c outputs rows
  {128c, 1024+128c}.
"""

import numpy as np
import ml_dtypes
from contextlib import ExitStack

import concourse.bass as bass
import concourse.bacc as bacc
import concourse.mybir as mybir
import concourse.tile as tile
from concourse.bass_utils import run_bass_kernel_spmd

F32 = mybir.dt.float32
BF16 = mybir.dt.bfloat16
AF = mybir.ActivationFunctionType

B, L, D, H, DK = 2, 2048, 1024, 16, 64
NCORES = 8
NH = 4              # heads per core
CPC = NH * DK       # 256 qkv cols per core
VA = NH * (DK + 1)  # 260, V-aug width (per-head [1 | V])
LT = L // 128       # 16 k-chunks
DCH = D // 128      # 8 d-chunks
NQ = 4              # l-quarters (projection granularity)
QW = L // NQ        # 512


def _emit(tc, nc, x, wq, bq, wk, bk, wv, bv, wo, bo, idm, out):
    with ExitStack() as es:
        # ---------------- persistent pools ----------------
        const = es.enter_context(tc.tile_pool(name="const", bufs=1))
        wq_sb = const.tile([128, DCH, CPC], BF16)
        wk_sb = const.tile([128, DCH, CPC], BF16)
        wv_sb = const.tile([128, DCH, VA], BF16)
        wo_sb = const.tile([128, DCH, D], BF16)
        bq_sb = const.tile([128, 2, 1], F32)
        bk_sb = const.tile([128, 2, 1], F32)
        bv_bc = const.tile([128, VA], F32)
        bo_bc = const.tile([128, D], F32)

        proj = es.enter_context(tc.tile_pool(name="proj", bufs=1))
        qt = proj.tile([128, 2, L], BF16)       # Q^T, c-chunk m rows
        kt = proj.tile([128, 2, L], BF16)       # K^T
        vaug = proj.tile([128, LT, VA], BF16)   # [1 | V] per k-chunk

        # ---------------- phase A: X^T + projections ----------------
        with ExitStack() as phA:
            xtp = phA.enter_context(tc.tile_pool(name="xtp", bufs=1))
            xt = xtp.tile([128, DCH, L], BF16)  # X^T (d on partitions)
            ident = xtp.tile([128, 128], BF16)
            xload = phA.enter_context(tc.tile_pool(name="xload", bufs=2))
            pst = phA.enter_context(tc.tile_pool(name="pst", bufs=2,
                                                 space="PSUM"))
            psA = phA.enter_context(tc.tile_pool(name="psA", bufs=3,
                                                 space="PSUM"))
            psV = phA.enter_context(tc.tile_pool(name="psV", bufs=2,
                                                 space="PSUM"))

            # identity (host-supplied) + Q/K weights first on the
            # scalar queue so transposes/proj are never weight-blocked.
            nc.scalar.dma_start(ident, idm.ap())
            nc.scalar.dma_start(
                wk_sb, wk.ap().rearrange("(dc p) c -> p dc c", p=128))
            nc.scalar.dma_start(
                wq_sb, wq.ap().rearrange("(dc p) c -> p dc c", p=128))
            nc.scalar.dma_start(
                wv_sb, wv.ap().rearrange("(dc p) c -> p dc c", p=128))
            nc.scalar.dma_start(
                bk_sb, bk.ap().rearrange("(m p) o -> p m o", p=128))
            nc.scalar.dma_start(
                bq_sb, bq.ap().rearrange("(m p) o -> p m o", p=128))
            # broadcast loads (stride-0 partition reads) on SWDGE
            bv_ap = bv.ap()
            nc.gpsimd.dma_start(
                bv_bc, bass.AP(tensor=bv_ap.tensor, offset=bv_ap.offset,
                               ap=[[0, 128]] + list(bv_ap.ap[1:])))
            bo_ap = bo.ap()
            nc.gpsimd.dma_start(
                bo_bc, bass.AP(tensor=bo_ap.tensor, offset=bo_ap.offset,
                               ap=[[0, 128]] + list(bo_ap.ap[1:])))

            # X^T via PE transposes, per 128-row chunk of X
            xap = x.ap()

            def emit_xt(lt):
                xrow = xload.tile([128, D], BF16, tag="xrow",
                                  name=f"xrow{lt}")
                nc.sync.dma_start(xrow, xap[lt * 128:(lt + 1) * 128, :])
                for dg in range(2):
                    pt = pst.tile([128, 512], BF16, tag="pt",
                                  name=f"pt{lt}{dg}")
                    for q in range(4):
                        dc = dg * 4 + q
                        nc.tensor.transpose(
                            pt[:, q * 128:(q + 1) * 128],
                            xrow[:, dc * 128:(dc + 1) * 128], ident)
                    nc.vector.tensor_copy(
                        xt[:, dg * 4:dg * 4 + 4, lt * 128:(lt + 1) * 128],
                        pt.rearrange("p (a b) -> p a b", a=4))

            # Wo is only needed for Y at the very end
            nc.scalar.dma_start(
                wo_sb, wo.ap().rearrange("(j p) n -> p j n", p=128))

            # K^T / Q^T projections per l-quarter (K first: attention's
            # lhsT).  Bias-add + bf16 cast on the otherwise-idle ACT.
            for lq in range(NQ):
                for lt in range(lq * 4, lq * 4 + 4):
                    emit_xt(lt)
                for w_sb, b_sb, dst in ((wk_sb, bk_sb, kt),
                                        (wq_sb, bq_sb, qt)):
                    for m in range(2):
                        pq = psA.tile([128, QW], F32, tag="pq")
                        for dc in range(DCH):
                            nc.tensor.matmul(
                                pq,
                                w_sb[:, dc, m * 128:(m + 1) * 128],
                                xt[:, dc, lq * QW:(lq + 1) * QW],
                                start=(dc == 0), stop=(dc == DCH - 1))
                        nc.scalar.activation(
                            dst[:, m, lq * QW:(lq + 1) * QW], pq,
                            AF.Identity, bias=b_sb[:, m, :])
                # V projection for this quarter's 4 k-chunks (DVE adds
                # the tensor-valued bias and casts to bf16)
                for lt in range(lq * 4, lq * 4 + 4):
                    pv = psV.tile([128, VA], F32, tag="pv")
                    for dc in range(DCH):
                        nc.tensor.matmul(
                            pv, xt[:, dc, lt * 128:(lt + 1) * 128],
                            wv_sb[:, dc, :],
                            start=(dc == 0), stop=(dc == DCH - 1))
                    nc.vector.tensor_add(vaug[:, lt, :], pv, bv_bc)

        # ---------------- phase B1: attention + exchange ----------------
        dramp = es.enter_context(tc.tile_pool(name="dramp", bufs=1,
                                              space="DRAM"))
        ain = dramp.tile([2, NCORES, CPC, 128], BF16)
        aout = dramp.tile([2, NCORES, CPC, 128], BF16)

        with ExitStack() as phB:
            psS = phB.enter_context(tc.tile_pool(name="psS", bufs=2,
                                                 space="PSUM"))
            psO = phB.enter_context(tc.tile_pool(name="psO", bufs=2,
                                                 space="PSUM"))
            upool = phB.enter_context(tc.tile_pool(name="upool", bufs=6))
            npool = phB.enter_context(tc.tile_pool(name="npool", bufs=2))

            for half in range(2):
                q0 = half * 1024
                for h in range(NH):
                    m, r0 = h // 2, (h % 2) * 64
                    otp = psO.tile([65, 1024], F32, tag="otp",
                                   name=f"otp{half}{h}")
                    for kti in range(LT):
                        sp = psS.tile([128, 1024], F32, tag="sp",
                                      name=f"sp{half}{h}{kti}")
                        for nn in range(2):
                            nc.tensor.matmul(
                                sp[:, nn * 512:(nn + 1) * 512],
                                kt[r0:r0 + 64, m,
                                   kti * 128:(kti + 1) * 128],
                                qt[r0:r0 + 64, m,
                                   q0 + nn * 512:q0 + (nn + 1) * 512],
                                start=True, stop=True)
                        u = upool.tile([128, 1024], BF16, tag="u",
                                       name=f"u{half}{h}{kti}")
                        nc.scalar.activation(u, sp, AF.Exp, scale=0.125)
                        for nn in range(2):
                            nc.tensor.matmul(
                                otp[:, nn * 512:(nn + 1) * 512],
                                vaug[:, kti, h * 65:(h + 1) * 65],
                                u[:, nn * 512:(nn + 1) * 512],
                                start=(kti == 0), stop=(kti == LT - 1))
                    # normalization: row 0 of otp is the denominator.
                    # recip on partition 0, gpsimd broadcast, multiply
                    # straight out of PSUM (evacuates otp).
                    rec1 = npool.tile([1, 1024], F32, tag="rec1",
                                      name=f"rec1{half}{h}")
                    nc.vector.reciprocal_approx_fast(rec1, otp[0:1, :])
                    rbc = npool.tile([65, 1024], F32, tag="rbc",
                                     name=f"rbc{half}{h}")
                    nc.gpsimd.partition_broadcast(rbc, rec1, channels=65)
                    otn = npool.tile([65, 1024], BF16, tag="otn",
                                     name=f"otn{half}{h}")
                    nc.vector.tensor_mul(otn, otp, rbc)
                    # pack into ain[half] (plain bf16 copy)
                    dst = ain[half, :, h * 64:(h + 1) * 64, :].rearrange(
                        "s p l -> p s l")
                    nc.sync.dma_start(
                        dst, otn[1:65, :].rearrange("p (s l) -> p s l",
                                                    s=NCORES))
                nc.gpsimd.collective_compute(
                    "AllToAll", mybir.AluOpType.bypass,
                    replica_groups=[list(range(NCORES))],
                    ins=[ain[half].opt()], outs=[aout[half].opt()])

        # ---------------- phase B2: output Y ----------------
        with ExitStack() as phC:
            psY = phC.enter_context(tc.tile_pool(name="psY", bufs=4,
                                                 space="PSUM"))
            opool = phC.enter_context(tc.tile_pool(name="opool", bufs=2))
            ypool = phC.enter_context(tc.tile_pool(name="ypool", bufs=2))
            outap = out.ap()
            for half in range(2):
                for b in range(B):
                    oall = opool.tile([128, DCH, 128], BF16, tag="oall",
                                      name=f"oall{half}{b}")
                    src = aout[half, 4 * b:4 * b + 4, :, :]
                    nc.sync.dma_start(
                        oall,
                        src.rearrange("s (jl p) l -> p (s jl) l", p=128))
                    y_sb = ypool.tile([128, D], F32, tag="ysb",
                                      name=f"ysb{half}{b}")
                    yp0 = psY.tile([128, 512], F32, tag="yp",
                                   name=f"yp{half}{b}0")
                    yp1 = psY.tile([128, 512], F32, tag="yp",
                                   name=f"yp{half}{b}1")
                    for j in range(DCH):
                        for nn, yp in ((0, yp0), (1, yp1)):
                            nc.tensor.matmul(
                                yp, oall[:, j, :],
                                wo_sb[:, j, nn * 512:(nn + 1) * 512],
                                start=(j == 0), stop=(j == DCH - 1))
                    for nn, yp in ((0, yp0), (1, yp1)):
                        nc.vector.tensor_add(
                            y_sb[:, nn * 512:(nn + 1) * 512], yp,
                            bo_bc[:, nn * 512:(nn + 1) * 512])
                    nc.sync.dma_start(outap[b, half, :, :], y_sb)


_CACHED_NC = None


def _build_program():
    global _CACHED_NC
    if _CACHED_NC is not None:
        return _CACHED_NC
    nc = bacc.Bacc(None, target_bir_lowering=False, debug=False,
                   num_devices=NCORES)
    x = nc.declare_dram_parameter("x", [L, D], BF16, isOutput=False)
    wq = nc.declare_dram_parameter("wq", [D, CPC], BF16, isOutput=False)
    bq = nc.declare_dram_parameter("bq", [CPC, 1], F32, isOutput=False)
    wk = nc.declare_dram_parameter("wk", [D, CPC], BF16, isOutput=False)
    bk = nc.declare_dram_parameter("bk", [CPC, 1], F32, isOutput=False)
    wv = nc.declare_dram_parameter("wv", [D, VA], BF16, isOutput=False)
    bv = nc.declare_dram_parameter("bv", [1, VA], F32, isOutput=False)
    wo = nc.declare_dram_parameter("wo", [D, D], BF16, isOutput=False)
    bo = nc.declare_dram_parameter("bo", [1, D], F32, isOutput=False)
    idm = nc.declare_dram_parameter("idm", [128, 128], BF16,
                                    isOutput=False)
    out = nc.declare_dram_parameter("out", [B, 2, 128, D], F32,
                                    isOutput=True)

    with tile.TileContext(nc) as tc:
        _emit(tc, nc, x, wq, bq, wk, bk, wv, bv, wo, bo, idm, out)
    nc.finalize()
    _CACHED_NC = nc
    return nc


def _make_in_maps(X, WQ, bQ, WK, bK, WV, bV, WO, bO):
    bf = ml_dtypes.bfloat16
    X = np.asarray(X, np.float32)
    WQ = np.asarray(WQ, np.float32)
    WK = np.asarray(WK, np.float32)
    WV = np.asarray(WV, np.float32)
    WO = np.ascontiguousarray(np.asarray(WO, np.float32).astype(bf))
    bO = np.asarray(bO, np.float32).reshape(1, D)
    in_maps = []
    for c in range(NCORES):
        b, g = c // 4, c % 4
        cs = slice(CPC * g, CPC * (g + 1))
        wva = np.zeros((D, VA), np.float32)
        bva = np.zeros((1, VA), np.float32)
        for h in range(NH):
            # ones column FIRST, then the 64 V columns
            bva[0, h * 65] = 1.0
            wva[:, h * 65 + 1:h * 65 + 65] = WV[:, CPC * g + 64 * h:
                                                CPC * g + 64 * (h + 1)]
            bva[0, h * 65 + 1:h * 65 + 65] = bV[CPC * g + 64 * h:
                                                CPC * g + 64 * (h + 1)]
        in_maps.append({
            "x": np.ascontiguousarray(X[b].astype(bf)),
            "wq": np.ascontiguousarray(WQ[:, cs].astype(bf)),
            "bq": np.ascontiguousarray(np.asarray(bQ, np.float32)[cs]
                                       .reshape(CPC, 1)),
            "wk": np.ascontiguousarray(WK[:, cs].astype(bf)),
            "bk": np.ascontiguousarray(np.asarray(bK, np.float32)[cs]
                                       .reshape(CPC, 1)),
            "wv": np.ascontiguousarray(wva.astype(bf)),
            "bv": bva,
            "wo": WO,
            "bo": np.ascontiguousarray(bO),
            "idm": np.eye(128, dtype=bf),
        })
    return in_maps


def _assemble(results):
    full = np.empty((B, L, D), np.float32)
    for c in range(NCORES):
        o = results[c]["out"]  # [B, 2, 128, D]
        for b in range(B):
            full[b, 128 * c:128 * (c + 1), :] = o[b, 0]
            full[b, 1024 + 128 * c:1024 + 128 * (c + 1), :] = o[b, 1]
    return full


def run(inputs, trace=False):
    nc = _build_program()
    in_maps = _make_in_maps(**inputs)
    res = run_bass_kernel_spmd(nc, in_maps, list(range(NCORES)), trace=trace)
    return _assemble(res.results), res


def kernel(X, WQ, bQ, WK, bK, WV, bV, WO, bO):
    out, _ = run(dict(X=X, WQ=WQ, bQ=bQ, WK=WK, bK=bK, WV=WV, bV=bV,
                      WO=WO, bO=bO))
    return out
